# revision 1
# baseline (speedup 1.0000x reference)
"""Bidirectional chamfer loss on 8 Trainium2 NeuronCores.

Problem: N=16384 render points (128x128x2), M=16384 contour points (16384x2),
output = sum_i min_j ||p_i - q_j|| + sum_j min_i ||p_i - q_j||  (scalar f32).

Strategy (retrieval_knn):
  - Host: sort p and q by x-coordinate. Core c gets the c-th slice of 2048
    sorted render points plus a window of W=4096 contiguous sorted contour
    points centered on the slice's x-range. For uniform points in a 512px
    image the window gives >=~32px of x-margin on each side, far beyond any
    nearest-neighbor distance, so windowed mins equal true mins. This is
    *certified exactly* on the host afterwards (excluded points are at least
    the window-edge x-distance away); any failing row/column falls back to
    an exact numpy computation, so the kernel is correct for any input.
  - Device (per core): tensor engine computes d2/16 blocks directly via a
    K=4 matmul (lhsT rows [px/4, py/4, p2/16, 1] x rhs rows
    [-2qx/4, -2qy/4, 1, q2/16]); both the (p-rows x q-cols) matrix and its
    transpose are produced so each min direction is a free-axis reduce_min.
    No cross-core communication: row mins are per-slice, column mins are
    per-window and host-combined with a scatter-min.
  - Host: sqrt + sums in float64, cast to float32.
"""

import numpy as np

# ---- hardcoded problem geometry (from the problem spec) ----
N = 16384            # render points (128*128)
M = 16384            # contour points
NCORES = 8
NP_CORE = N // NCORES          # 2048 render points per core
W = 4096                       # contour window per core
P = 128                        # partitions
IT_A = NP_CORE // P            # 16 i-tiles (matrix A)
IT_B = W // P                  # 32 j-tiles (matrix B)
CHUNK = 2048                   # psum chunk free size (4 banks)
MMF = 512                      # fp32 matmul max moving free dim

_COMPILED = {}


def _build_program():
    """Build the SPMD bass program (same program for all 8 cores).

    Raw bass (not Tile): the pipeline is a simple PE->DVE double buffer and
    Tile's semaphore pass emits 2 waits on the first matmul of a reused PSUM
    slot, which walrus can't encode (1 wait slot per instruction). With
    explicit Block bodies every wait is a standalone instruction.
    """
    import concourse.bass as bass
    from concourse import mybir

    f32 = mybir.dt.float32
    X = mybir.AxisListType.X
    MIN = mybir.AluOpType.min

    nc = bass.Bass("TRN2", target_bir_lowering=False, debug=False,
                   num_devices=NCORES)

    TOT = NP_CORE + W + W + NP_CORE   # 12288
    inp = nc.dram_tensor("inp", [4, TOT], f32, kind="ExternalInput").ap()
    rowout = nc.dram_tensor("rowout", [P, IT_A], f32, kind="ExternalOutput").ap()
    colout = nc.dram_tensor("colout", [P, IT_B], f32, kind="ExternalOutput").ap()

    o0, o1, o2 = NP_CORE, NP_CORE + W, NP_CORE + 2 * W
    NCH_A = IT_A * (W // CHUNK)       # 32 chunks for matrix A
    NCH_B = IT_B                      # 32 chunks for matrix B (CHUNK == NP_CORE)
    NCH = NCH_A + NCH_B

    with (
        nc.sbuf_tensor([128, TOT], f32) as t_inp,
        nc.sbuf_tensor([P, NCH_A], f32) as accA,
        nc.sbuf_tensor([P, NCH_B], f32) as accB,
        nc.sbuf_tensor([P, IT_A], f32) as rmin,
        nc.psum_tensor([P, CHUNK], f32) as ps0,
        nc.psum_tensor([P, CHUNK], f32) as ps1,
        nc.semaphore() as dma_sem,
        nc.semaphore() as pe_sem,
        nc.semaphore() as dve_sem,
        nc.Block() as block,
    ):
        def chunk_aps(k):
            """([4 lhsT APs], [4 rhs APs], accum AP) for chunk k.

            The m-th matmul of a chunk reads its K=4 operands from the
            replica at partitions 32m..32m+3 and runs in PE row group 32m,
            so all 4 matmuls of a chunk execute concurrently in the array.
            """
            if k < NCH_A:
                t, h = divmod(k, W // CHUNK)
                lc = slice(t * P, (t + 1) * P)
                rc = [slice(o0 + h * CHUNK + m * MMF, o0 + h * CHUNK + (m + 1) * MMF)
                      for m in range(CHUNK // MMF)]
                acc = accA[:, k:k + 1]
            else:
                u = k - NCH_A
                lc = slice(o1 + u * P, o1 + (u + 1) * P)
                rc = [slice(o2 + m * MMF, o2 + (m + 1) * MMF)
                      for m in range(CHUNK // MMF)]
                acc = accB[:, u:u + 1]
            lhsT = [t_inp[32 * m:32 * m + 4, lc] for m in range(CHUNK // MMF)]
            rhs = [t_inp[32 * m:32 * m + 4, rc[m]] for m in range(CHUNK // MMF)]
            return lhsT, rhs, acc

        @block.sync
        def _(sync):
            # replicate the [4, TOT] operand block into PE row groups
            # 0/32/64/96 so K=4 matmuls can pack 4-wide in the array
            for r in (0, 32, 64, 96):
                sync.dma_start(t_inp[r:r + 4, :], inp).then_inc(dma_sem, 16)
            sync.wait_ge(dve_sem, NCH + 1)
            sync.dma_start(rowout, rmin[:]).then_inc(dma_sem, 16)
            sync.dma_start(colout, accB[:]).then_inc(dma_sem, 16)

        @block.tensor
        def _(pe):
            pe.wait_ge(dma_sem, 64)
            for k in range(NCH):
                ps = ps0 if k % 2 == 0 else ps1
                if k >= 2:
                    pe.wait_ge(dve_sem, k - 1)  # slot's previous reduce done
                lhsT, rhs, _ = chunk_aps(k)
                last = None
                for m in range(CHUNK // MMF):
                    last = nc.tensor.matmul(
                        ps[:, m * MMF:(m + 1) * MMF], lhsT[m], rhs[m],
                        start=True, stop=True,
                        tile_position=(32 * m, 0),
                    )
                last.then_inc(pe_sem, 1)

        @block.vector
        def _(vector):
            for k in range(NCH):
                ps = ps0 if k % 2 == 0 else ps1
                vector.wait_ge(pe_sem, k + 1)
                _, _, acc = chunk_aps(k)
                nc.vector.tensor_reduce(
                    acc, ps[:], axis=X, op=MIN,
                ).then_inc(dve_sem, 1)
            # combine the window-halves of each i-tile: [P, 16, 2] -> [P, 16]
            nc.vector.tensor_reduce(
                rmin[:], accA[:].rearrange("p (t h) -> p t h", h=2),
                axis=X, op=MIN,
            ).then_inc(dve_sem, 1)

    return nc


def _get_program():
    if "nc" not in _COMPILED:
        _COMPILED["nc"] = _build_program()
    return _COMPILED["nc"]


def _prep(points, scale_sq):
    """rows [x/4, y/4, sq/16-or-1, 1-or-sq/16] for the K=4 matmul.

    Coordinates deliberately NOT centered: keeping the same term magnitudes
    as the reference's p2+q2-2pq makes our fp32 rounding errors correlate
    with the reference's, minimizing the deviation from its fp32 output
    (measured 7.4e-5 uncentered vs 9.8e-5 centered).
    """
    x = points[:, 0].astype(np.float32)
    y = points[:, 1].astype(np.float32)
    sq = (x * x + y * y) / np.float32(16.0)
    ones = np.ones_like(x)
    if scale_sq == "lhsT":   # stationary side: [x/4, y/4, sq/16, 1]
        return np.stack([x / 4.0, y / 4.0, sq, ones]).astype(np.float32)
    else:                     # moving side: [-2x/4, -2y/4, 1, sq/16]
        return np.stack([-x / 2.0, -y / 2.0, ones, sq]).astype(np.float32)


def _make_in_maps(p: np.ndarray, q: np.ndarray):
    """Sort by x, slice/window per core, build device operands."""
    po = np.argsort(p[:, 0], kind="stable")
    qo = np.argsort(q[:, 0], kind="stable")
    ps = p[po]
    qs = q[qo]
    qx = qs[:, 0]

    in_maps = []
    starts = []
    for c in range(NCORES):
        sl = ps[c * NP_CORE:(c + 1) * NP_CORE]
        s_lo = np.searchsorted(qx, sl[0, 0])
        s_hi = np.searchsorted(qx, sl[-1, 0])
        start = int(np.clip((s_lo + s_hi) // 2 - W // 2, 0, M - W))
        starts.append(start)
        qw = qs[start:start + W]
        inp = np.concatenate([
            _prep(sl, "lhsT"), _prep(qw, "rhs"),
            _prep(qw, "lhsT"), _prep(sl, "rhs"),
        ], axis=1)
        in_maps.append({"inp": np.ascontiguousarray(inp)})
    return in_maps, starts, ps, qs


def kernel(img_render_points: np.ndarray, contour_points: np.ndarray) -> np.ndarray:
    # NOTE: do not enable jax_compilation_cache_dir here — loading this
    # program from the jax persistent cache produces executables that fail
    # with NRT_EXEC_UNIT_UNRECOVERABLE on the axon PJRT path. The NEFF
    # compile itself is cached by the environment's own compile cache.
    from concourse.bass_utils import run_bass_kernel_spmd

    p = np.asarray(img_render_points, dtype=np.float32).reshape(-1, 2)
    q = np.asarray(contour_points, dtype=np.float32)
    assert p.shape == (N, 2) and q.shape == (M, 2)

    in_maps, starts, ps, qs = _make_in_maps(p, q)
    qx = qs[:, 0]

    nc = _get_program()
    res = run_bass_kernel_spmd(nc, in_maps, list(range(NCORES)))
    results = res.results

    # ---- host combine ----
    rowmin2 = np.empty(N, dtype=np.float64)   # d2, sorted-p order
    colmin2 = np.full(M, np.inf, dtype=np.float64)  # d2, sorted-q order
    for c in range(NCORES):
        ro = np.asarray(results[c]["rowout"], dtype=np.float64) * 16.0  # [P, IT_A]
        co = np.asarray(results[c]["colout"], dtype=np.float64) * 16.0  # [P, IT_B]
        # rowout[p, t] -> sorted index c*NP_CORE + t*P + p
        rowmin2[c * NP_CORE:(c + 1) * NP_CORE] = ro.T.reshape(-1)
        # colout[p, u] -> window-local j = u*P + p
        w = co.T.reshape(-1)
        seg = slice(starts[c], starts[c] + W)
        np.minimum.at(colmin2, seg, w)

    # ---- exact certification of the windowing ----
    px = ps[:, 0].astype(np.float64)
    qxd = qx.astype(np.float64)
    # rows: excluded contour points are beyond the window edges in x
    row_bound = np.full(N, np.inf)
    for c in range(NCORES):
        s = starts[c]
        idx = slice(c * NP_CORE, (c + 1) * NP_CORE)
        b = np.full(NP_CORE, np.inf)
        if s > 0:
            b = np.minimum(b, np.maximum(px[idx] - qxd[s - 1], 0.0) ** 2)
        if s + W < M:
            b = np.minimum(b, np.maximum(qxd[s + W] - px[idx], 0.0) ** 2)
        row_bound[idx] = b
    bad_rows = np.nonzero(rowmin2 > row_bound)[0]

    # cols: for each contour point, cores that excluded it are at least
    # the x-distance to that core's p-slice away
    col_bound = np.full(M, np.inf)
    for c in range(NCORES):
        s = starts[c]
        pmin = px[c * NP_CORE]
        pmax = px[(c + 1) * NP_CORE - 1]
        d = np.maximum(np.maximum(pmin - qxd, qxd - pmax), 0.0) ** 2
        excl = np.ones(M, dtype=bool)
        excl[s:s + W] = False
        col_bound[excl] = np.minimum(col_bound[excl], d[excl])
    bad_cols = np.nonzero(colmin2 > col_bound)[0]

    # ---- exact numpy fallback for any uncertified entries ----
    if bad_rows.size:
        pp = ps[bad_rows].astype(np.float64)
        qq = qs.astype(np.float64)
        d2 = ((pp[:, None, :] - qq[None, :, :]) ** 2).sum(-1)
        rowmin2[bad_rows] = d2.min(axis=1)
    if bad_cols.size:
        qq = qs[bad_cols].astype(np.float64)
        pp = ps.astype(np.float64)
        d2 = ((qq[:, None, :] - pp[None, :, :]) ** 2).sum(-1)
        colmin2[bad_cols] = d2.min(axis=1)

    total = (np.sqrt(np.maximum(rowmin2, 0.0)).sum()
             + np.sqrt(np.maximum(colmin2, 0.0)).sum())
    return np.float32(total)

